# revision 15
# baseline (speedup 1.0000x reference)
"""MoE layer (B=4, N=2048, C=1024, F=4096, E=8, top-2) on 8 trn2 NeuronCores.

Sharding: expert-parallel. The host computes the (tiny, ~0.1% of FLOPs)
router and dispatches each expert's tokens to one core as part of sharding;
each core runs its expert's full FFN  relu(xg @ w1.T + b1) @ w2.T + b2,
gated by the combine weight, over its gathered tokens.  The host combine
scatter-adds the per-expert results back into the full output.

Fast path (b1 == b2 == 0, the spec'd fill): tokens are pre-gated on host,
all matmul operands are bf16 (PE runs bf16 at the same 1 col/cycle rate as
f32r but with Fast Weight Load, and DMA bytes halve); accumulation stays
fp32 in PSUM.  End-to-end rel err ~3e-3 vs the fp32 reference.
"""

import numpy as np
import ml_dtypes

P = 128
C = 1024
F = 4096
E = 8
SCH = 384  # token chunk: 3 PSUM banks (x 2 C-halves) for y + 2 for h = 8
BF16 = ml_dtypes.bfloat16


def _build(cap: int):
    import concourse.mybir as mybir
    from concourse import bacc
    from concourse.tile import TileContext

    f32 = mybir.dt.float32
    f32r = mybir.dt.float32r
    nS = cap // SCH
    nc = bacc.Bacc(None, target_bir_lowering=False)

    xgT = nc.dram_tensor("xgT", [C, cap], f32, kind="ExternalInput")
    w1t = nc.dram_tensor("w1t", [C, F], f32, kind="ExternalInput")
    w2t = nc.dram_tensor("w2t", [F, C], f32, kind="ExternalInput")
    b1r = nc.dram_tensor("b1r", [P, F // P], f32, kind="ExternalInput")
    b2r = nc.dram_tensor("b2r", [P, C], f32, kind="ExternalInput")
    wg = nc.dram_tensor("wg", [P, cap // P], f32, kind="ExternalInput")
    yg = nc.dram_tensor("yg", [cap, C], f32, kind="ExternalOutput")

    w1v = w1t.ap().rearrange("(co ci) f -> ci co f", ci=P)  # [128, 8, F]
    xgv = xgT.ap().rearrange("(co ci) n -> ci co n", ci=P)  # [128, 8, cap]

    with TileContext(nc) as tc:
        with (
            tc.tile_pool(name="consts", bufs=1) as consts,
            tc.tile_pool(name="wpool", bufs=4) as wpool,
            tc.tile_pool(name="xpool", bufs=2) as xpool,
            tc.tile_pool(name="hpool", bufs=3) as hpool,
            tc.tile_pool(name="ypool", bufs=3) as ypool,
            tc.tile_pool(name="psum_h", bufs=2, space="PSUM") as psum_h,
            tc.tile_pool(name="psum_y", bufs=1, space="PSUM") as psum_y,
        ):
            b1_sb = consts.tile([P, F // P], f32)
            nc.sync.dma_start(b1_sb[:], b1r[:, :])
            b2_sb = consts.tile([P, C], f32)
            nc.sync.dma_start(b2_sb[:], b2r[:, :])
            wg_sb = consts.tile([P, cap // P], f32)
            nc.sync.dma_start(wg_sb[:], wg[:, :])

            for s in range(nS):
                xg_s = xpool.tile([P, 8, SCH], f32r, tag="xg")
                nc.sync.dma_start(xg_s[:], xgv[:, :, s * SCH : (s + 1) * SCH].bitcast(f32r))

                yps = [
                    [
                        psum_y.tile(
                            [P, 512], f32, tag=f"y_{t}_{cc}", name=f"y_{t}_{cc}"
                        )
                        for cc in range(2)
                    ]
                    for t in range(3)
                ]

                for f in range(F // P):  # 32
                    w1c = wpool.tile([P, 8, P], f32r, tag="w1c")
                    nc.sync.dma_start(w1c[:], w1v[:, :, f * P : (f + 1) * P].bitcast(f32r))
                    w2c = wpool.tile([P, C], f32r, tag="w2c")
                    nc.sync.dma_start(w2c[:], w2t[f * P : (f + 1) * P, :].bitcast(f32r))

                    hps = psum_h.tile([P, SCH], f32, tag="h")
                    for c in range(8):
                        nc.tensor.matmul(
                            hps[:],
                            lhsT=w1c[:, c, :],
                            rhs=xg_s[:, c, :],
                            start=(c == 0),
                            stop=(c == 7),
                        )
                    hT = hpool.tile([P, SCH], f32r, tag="hT")
                    nc.scalar.activation(
                        hT[:],
                        hps[:],
                        mybir.ActivationFunctionType.Relu,
                        bias=b1_sb[:, f : f + 1],
                        scale=1.0,
                    )
                    for t in range(3):
                        for cc in range(2):
                            nc.tensor.matmul(
                                yps[t][cc][:],
                                lhsT=hT[:, t * P : (t + 1) * P],
                                rhs=w2c[:, cc * 512 : (cc + 1) * 512],
                                start=(f == 0),
                                stop=(f == F // P - 1),
                            )

                for t in range(3):
                    y_sb = ypool.tile([P, C], f32, tag="y_sb")
                    for cc in range(2):
                        sl = slice(cc * 512, (cc + 1) * 512)
                        nc.vector.tensor_add(y_sb[:, sl], yps[t][cc][:], b2_sb[:, sl])
                    yf = ypool.tile([P, C], f32, tag="yf")
                    nc.scalar.mul(yf[:], y_sb[:], wg_sb[:, s * 3 + t : s * 3 + t + 1])
                    nc.sync.dma_start(
                        yg[(s * 3 + t) * P : (s * 3 + t + 1) * P, :], yf[:]
                    )
    nc.compile()
    return nc




def _chunks(cap):
    """384-token chunks (fewest matmuls under the 8-PSUM-bank budget); the
    runt goes last so the tail retire chain is short."""
    sizes = [384] * (cap // 384)
    rem = cap - 384 * len(sizes)
    if rem:
        sizes.append(rem)
    return sizes


def _build_fast(cap: int):
    """Fast path (b1 == 0 and b2 == 0): inputs pre-gated and pre-tiled on host.

    All matmul operands bf16 (fp32 PSUM accumulation).  f-groups (NF_G
    chunks of F) outer, token chunks inner; weights stream through SBUF
    once, per-chunk y accumulates fp32 in SBUF across groups.
      inputs : xgf [cap*1024]  gated tokens bf16, per-chunk tiled [ci, co, n]
               w1p [32, 128, 8, 128]  w1.T tiled for mm1 lhsT, bf16
               w2t [4096, 1024] bf16
      output : yg  [cap, 1024] fp32
    """
    import concourse.mybir as mybir
    from concourse import bacc
    from concourse.tile import TileContext

    f32 = mybir.dt.float32
    bf16 = mybir.dt.bfloat16
    sizes = _chunks(cap)
    offs = [sum(sizes[:i]) for i in range(len(sizes))]
    NF_G = 4
    NG = (F // P) // NF_G  # 8 groups
    nc = bacc.Bacc(None, target_bir_lowering=False)

    xgf = nc.dram_tensor("xgf", [cap * C], bf16, kind="ExternalInput")
    w1p = nc.dram_tensor("w1p", [F // P, P, 8, P], bf16, kind="ExternalInput")
    w2t = nc.dram_tensor("w2t", [F, C], bf16, kind="ExternalInput")
    yg = nc.dram_tensor("yg", [cap, C], f32, kind="ExternalOutput")

    with TileContext(nc) as tc:
        with (
            tc.tile_pool(name="ybuf", bufs=1) as ybuf,
            tc.tile_pool(name="wpool", bufs=2) as wpool,
            tc.tile_pool(name="xpool", bufs=2) as xpool,
            tc.tile_pool(name="hpool", bufs=3) as hpool,
            tc.tile_pool(name="warm", bufs=1) as warm,
            tc.tile_pool(name="psum_h", bufs=2, space="PSUM") as psum_h,
            tc.tile_pool(name="psum_y", bufs=1, space="PSUM") as psum_y,
        ):
            # HAM warmup: a burst of tiny matmuls on a zeroed scratch tile
            # keeps the PE busy while the first x/w DMAs are in flight, so
            # the clock gate lifts (1.2 -> 2.4 GHz) before real work starts
            wsb = warm.tile([P, 64], bf16, name="wsb", tag="wsb")
            nc.gpsimd.memset(wsb[:], 0)
            wps = psum_h.tile([P, 384], f32, tag="h", name="hps")
            for _ in range(76):
                nc.tensor.matmul(
                    wps[:32, :64], lhsT=wsb[:, :32], rhs=wsb[:, :64],
                    start=True, stop=True,
                )
            y_all = [
                [
                    ybuf.tile([P, C], f32, name=f"yall_{s}_{t}", tag=f"yall_{s}_{t}")
                    for t in range((sz + P - 1) // P)
                ]
                for s, sz in enumerate(sizes)
            ]

            def load_xg(s):
                sz = sizes[s]
                xg_s = xpool.tile([P, 8, sz], bf16, tag="xg", name="xg_s")
                src = xgf[offs[s] * C : (offs[s] + sz) * C]
                v = src.rearrange("(ci co n) -> ci co n", ci=P, co=8)
                nc.sync.dma_start(xg_s[:], v)
                return xg_s

            yq = 0  # rotating PSUM-y slot cursor (3 slots of 2 banks)
            for g in range(NG):
                w1g = wpool.tile([P, NF_G, 8, P], bf16, tag="w1g", name="w1g")
                w2g = wpool.tile([P, NF_G, C], bf16, tag="w2g", name="w2g")

                def load_w1(fl, g=g, w1g=w1g):
                    nc.sync.dma_start(w1g[:, fl], w1p[g * NF_G + fl])

                def load_w2(fl, g=g, w2g=w2g):
                    r0 = (g * NF_G + fl) * P
                    nc.sync.dma_start(w2g[:, fl], w2t[r0 : r0 + P, :])

                if g == 0:
                    # startup: the DMA engines round-robin across in-flight
                    # TRANSFERS, so bandwidth share ~ number of sub-DMAs.
                    # Split the critical first transfers (w1 fl0 into 2, the
                    # first x chunk into 4) so they complete first, then
                    # dispatch the rest in pipeline-consumption order.
                    nc.sync.dma_start(w1g[:, 0, :4], w1p[0][:, :4])
                    nc.sync.dma_start(w1g[:, 0, 4:], w1p[0][:, 4:])
                    xg_next = xpool.tile(
                        [P, 8, sizes[0]], bf16, tag="xg", name="xg_s"
                    )
                    v0 = xgf[0 : sizes[0] * C].rearrange(
                        "(ci co n) -> ci co n", ci=P, co=8
                    )
                    for qq in range(4):
                        nc.sync.dma_start(
                            xg_next[:, 2 * qq : 2 * qq + 2, :],
                            v0[:, 2 * qq : 2 * qq + 2, :],
                        )
                    load_w1(1)
                    load_w2(0)
                    load_w1(2)
                    load_w2(1)
                    load_w1(3)
                    load_w2(2)
                    load_w2(3)
                else:
                    for fl in range(NF_G):
                        load_w1(fl)
                    for fl in range(NF_G):
                        load_w2(fl)

                for s, sz in enumerate(sizes):
                    nt = (sz + P - 1) // P
                    xg_s = xg_next
                    # prefetch the next chunk (wraps to s=0 of the next group)
                    if s + 1 < len(sizes):
                        xg_next = load_xg(s + 1)
                    elif g + 1 < NG:
                        xg_next = load_xg(0)

                    # rotate PSUM-y slots so consecutive chunks hit different
                    # banks (the next chunk's first mm2 must not wait on this
                    # chunk's retire reads)
                    yps = [
                        psum_y.tile(
                            [P, C], f32,
                            tag=f"y_{(yq + t) % 3}", name=f"y_{(yq + t) % 3}",
                        )
                        for t in range(nt)
                    ]
                    yq += nt

                    def mm2(fl, hT, yps=yps, nt=nt, w2g=w2g):
                        for t in range(nt):
                            for cc in range(2):
                                nc.tensor.matmul(
                                    yps[t][:, cc * 512 : (cc + 1) * 512],
                                    lhsT=hT[:, t * P : (t + 1) * P],
                                    rhs=w2g[:, fl, cc * 512 : (cc + 1) * 512],
                                    start=(fl == 0),
                                    stop=(fl == NF_G - 1),
                                )

                    last_chunk = s == len(sizes) - 1

                    def retire(s=s, g=g, yps=yps, nt=nt, last_chunk=last_chunk):
                        if g == NG - 1 and last_chunk:
                            # kernel tail: split adds per C-half and alternate
                            # DMA queues so stores pipeline with the adds
                            for t in range(nt):
                                ya = y_all[s][t]
                                w0 = offs[s] // P + t
                                for hh in range(2):
                                    sl = slice(hh * 512, (hh + 1) * 512)
                                    nc.vector.tensor_add(
                                        ya[:, sl], ya[:, sl], yps[t][:, sl]
                                    )
                                    eng = nc.scalar if (2 * t + hh) % 2 else nc.sync
                                    eng.dma_start(
                                        yg[w0 * P : (w0 + 1) * P, sl], ya[:, sl]
                                    )
                            return
                        for t in range(nt):
                            ya = y_all[s][t]
                            if g == 0:
                                nc.vector.tensor_copy(ya[:], yps[t][:])
                            else:
                                nc.vector.tensor_add(ya[:], ya[:], yps[t][:])
                            if g == NG - 1:
                                w0 = offs[s] // P + t
                                eng = nc.scalar if t % 2 else nc.sync
                                eng.dma_start(yg[w0 * P : (w0 + 1) * P, :], ya[:])

                    # software pipeline: mm2 runs one fl behind mm1 (relu
                    # latency covered by the next fl's mm1s); the last mm2 +
                    # the PSUM retires of each chunk drain after the next
                    # chunk's first mm1 block
                    hT_prev = None
                    for fl in range(NF_G):
                        hps = psum_h.tile([P, 384], f32, tag="h", name="hps")
                        for c in range(8):
                            nc.tensor.matmul(
                                hps[:, :sz],
                                lhsT=w1g[:, fl, c, :],
                                rhs=xg_s[:, c, :],
                                start=(c == 0),
                                stop=(c == 7),
                            )
                        hT = hpool.tile([P, 384], bf16, tag="hT", name="hT")
                        if fl == NF_G - 1:
                            # last fl: per-token-tile relu so mm2(t) can
                            # start as soon as its slice is ready
                            for t in range(nt):
                                tl = slice(t * P, min((t + 1) * P, sz))
                                nc.scalar.activation(
                                    hT[:, tl],
                                    hps[:, tl],
                                    mybir.ActivationFunctionType.Relu,
                                )
                        else:
                            nc.scalar.activation(
                                hT[:, :sz],
                                hps[:, :sz],
                                mybir.ActivationFunctionType.Relu,
                            )
                        if hT_prev is not None:
                            mm2(fl - 1, hT_prev)
                        hT_prev = hT
                    mm2(NF_G - 1, hT_prev)
                    retire()
    nc.compile()
    return nc


_CACHE = {}
_TRACE = False  # test harness sets True to capture an NTFF profile
_LAST_RES = None


def _get_nc(cap, fast):
    key = (cap, fast)
    if key not in _CACHE:
        _CACHE[key] = _build_fast(cap) if fast else _build(cap)
    return _CACHE[key]


def _route(x_flat, router_w):
    """Top-2 routing, float64 for stable selection. Returns idx/weights per expert."""
    logits = x_flat.astype(np.float64) @ router_w.astype(np.float64).T
    t = np.exp(logits - logits.max(-1, keepdims=True))
    p = t / t.sum(-1, keepdims=True)
    top2 = np.argsort(-p, axis=-1)[:, :2]
    pv = np.take_along_axis(p, top2, axis=-1)
    wn = pv / (pv.sum(-1, keepdims=True) + 1e-9)
    return top2, wn


def kernel(x, router_w, w1, b1, w2, b2):
    from concourse.bass_utils import run_bass_kernel_spmd

    Bx, Nx, Cx = x.shape
    x_flat = np.ascontiguousarray(x.reshape(-1, Cx))
    T = x_flat.shape[0]

    top2, wn = _route(x_flat, router_w)
    idxs, gates = [], []
    for e in range(E):
        sel = top2 == e
        we = np.where(sel, wn, 0.0).sum(-1)
        idx = np.nonzero(sel.any(-1))[0]
        idxs.append(idx)
        gates.append(we[idx].astype(np.float32))
    cap = max(len(i) for i in idxs)
    fastcap = ((cap + P - 1) // P) * P
    cap = ((cap + SCH - 1) // SCH) * SCH

    fast = bool(np.all(b1 == 0) and np.all(b2 == 0))
    if fast:
        cap = fastcap
    nc = _get_nc(cap, fast)

    in_maps = []
    for e in range(E):
        n_e = len(idxs[e])
        xg = np.zeros((cap, Cx), np.float32)
        xg[:n_e] = x_flat[idxs[e]]
        wg = np.zeros(cap, np.float32)
        wg[:n_e] = gates[e]
        if fast:
            xg *= wg[:, None]  # pre-gate: exact since b1 == 0 and wg >= 0
            sizes = _chunks(cap)
            blocks, off = [], 0
            for sz in sizes:
                blocks.append(
                    np.ascontiguousarray(
                        xg[off : off + sz].reshape(sz, 8, P).transpose(2, 1, 0)
                    ).ravel()
                )
                off += sz
            in_maps.append(
                {
                    "xgf": np.concatenate(blocks).astype(BF16),
                    "w1p": np.ascontiguousarray(
                        w1[e].reshape(F // P, P, 8, P).transpose(0, 3, 2, 1)
                    ).astype(BF16),
                    "w2t": np.ascontiguousarray(w2[e].T).astype(BF16),
                }
            )
        else:
            in_maps.append(
                {
                    "xgT": np.ascontiguousarray(xg.T),
                    "w1t": np.ascontiguousarray(w1[e].T),
                    "w2t": np.ascontiguousarray(w2[e].T),
                    "b1r": np.ascontiguousarray(b1[e].reshape(F // P, P).T),
                    "b2r": np.ascontiguousarray(np.broadcast_to(b2[e], (P, Cx))),
                    "wg": np.ascontiguousarray(wg.reshape(cap // P, P).T),
                }
            )

    global _LAST_RES
    res = run_bass_kernel_spmd(nc, in_maps, core_ids=list(range(E)), trace=_TRACE)
    _LAST_RES = res

    out = np.zeros((T, Cx), np.float32)
    for e in range(E):
        n_e = len(idxs[e])
        out[idxs[e]] += res.results[e]["yg"][:n_e]
    return out.reshape(Bx, Nx, Cx)


# revision 21
# speedup vs baseline: 1.0022x; 1.0022x over previous
"""MoE layer (B=4, N=2048, C=1024, F=4096, E=8, top-2) on 8 trn2 NeuronCores.

Sharding: expert-parallel. The host computes the (tiny, ~0.1% of FLOPs)
router and dispatches each expert's tokens to one core as part of sharding;
each core runs its expert's full FFN  relu(xg @ w1.T + b1) @ w2.T + b2,
gated by the combine weight, over its gathered tokens.  The host combine
scatter-adds the per-expert results back into the full output.

Fast path (b1 == b2 == 0, the spec'd fill): tokens are pre-gated on host,
all matmul operands are bf16 (PE runs bf16 at the same 1 col/cycle rate as
f32r but with Fast Weight Load, and DMA bytes halve); accumulation stays
fp32 in PSUM.  End-to-end rel err ~3e-3 vs the fp32 reference.
"""

import numpy as np
import ml_dtypes

P = 128
C = 1024
F = 4096
E = 8
SCH = 384  # token chunk: 3 PSUM banks (x 2 C-halves) for y + 2 for h = 8
BF16 = ml_dtypes.bfloat16


def _build(cap: int):
    import concourse.mybir as mybir
    from concourse import bacc
    from concourse.tile import TileContext

    f32 = mybir.dt.float32
    f32r = mybir.dt.float32r
    nS = cap // SCH
    nc = bacc.Bacc(None, target_bir_lowering=False)

    xgT = nc.dram_tensor("xgT", [C, cap], f32, kind="ExternalInput")
    w1t = nc.dram_tensor("w1t", [C, F], f32, kind="ExternalInput")
    w2t = nc.dram_tensor("w2t", [F, C], f32, kind="ExternalInput")
    b1r = nc.dram_tensor("b1r", [P, F // P], f32, kind="ExternalInput")
    b2r = nc.dram_tensor("b2r", [P, C], f32, kind="ExternalInput")
    wg = nc.dram_tensor("wg", [P, cap // P], f32, kind="ExternalInput")
    yg = nc.dram_tensor("yg", [cap, C], f32, kind="ExternalOutput")

    w1v = w1t.ap().rearrange("(co ci) f -> ci co f", ci=P)  # [128, 8, F]
    xgv = xgT.ap().rearrange("(co ci) n -> ci co n", ci=P)  # [128, 8, cap]

    with TileContext(nc) as tc:
        with (
            tc.tile_pool(name="consts", bufs=1) as consts,
            tc.tile_pool(name="wpool", bufs=4) as wpool,
            tc.tile_pool(name="xpool", bufs=2) as xpool,
            tc.tile_pool(name="hpool", bufs=3) as hpool,
            tc.tile_pool(name="ypool", bufs=3) as ypool,
            tc.tile_pool(name="psum_h", bufs=2, space="PSUM") as psum_h,
            tc.tile_pool(name="psum_y", bufs=1, space="PSUM") as psum_y,
        ):
            b1_sb = consts.tile([P, F // P], f32)
            nc.sync.dma_start(b1_sb[:], b1r[:, :])
            b2_sb = consts.tile([P, C], f32)
            nc.sync.dma_start(b2_sb[:], b2r[:, :])
            wg_sb = consts.tile([P, cap // P], f32)
            nc.sync.dma_start(wg_sb[:], wg[:, :])

            for s in range(nS):
                xg_s = xpool.tile([P, 8, SCH], f32r, tag="xg")
                nc.sync.dma_start(xg_s[:], xgv[:, :, s * SCH : (s + 1) * SCH].bitcast(f32r))

                yps = [
                    [
                        psum_y.tile(
                            [P, 512], f32, tag=f"y_{t}_{cc}", name=f"y_{t}_{cc}"
                        )
                        for cc in range(2)
                    ]
                    for t in range(3)
                ]

                for f in range(F // P):  # 32
                    w1c = wpool.tile([P, 8, P], f32r, tag="w1c")
                    nc.sync.dma_start(w1c[:], w1v[:, :, f * P : (f + 1) * P].bitcast(f32r))
                    w2c = wpool.tile([P, C], f32r, tag="w2c")
                    nc.sync.dma_start(w2c[:], w2t[f * P : (f + 1) * P, :].bitcast(f32r))

                    hps = psum_h.tile([P, SCH], f32, tag="h")
                    for c in range(8):
                        nc.tensor.matmul(
                            hps[:],
                            lhsT=w1c[:, c, :],
                            rhs=xg_s[:, c, :],
                            start=(c == 0),
                            stop=(c == 7),
                        )
                    hT = hpool.tile([P, SCH], f32r, tag="hT")
                    nc.scalar.activation(
                        hT[:],
                        hps[:],
                        mybir.ActivationFunctionType.Relu,
                        bias=b1_sb[:, f : f + 1],
                        scale=1.0,
                    )
                    for t in range(3):
                        for cc in range(2):
                            nc.tensor.matmul(
                                yps[t][cc][:],
                                lhsT=hT[:, t * P : (t + 1) * P],
                                rhs=w2c[:, cc * 512 : (cc + 1) * 512],
                                start=(f == 0),
                                stop=(f == F // P - 1),
                            )

                for t in range(3):
                    y_sb = ypool.tile([P, C], f32, tag="y_sb")
                    for cc in range(2):
                        sl = slice(cc * 512, (cc + 1) * 512)
                        nc.vector.tensor_add(y_sb[:, sl], yps[t][cc][:], b2_sb[:, sl])
                    yf = ypool.tile([P, C], f32, tag="yf")
                    nc.scalar.mul(yf[:], y_sb[:], wg_sb[:, s * 3 + t : s * 3 + t + 1])
                    nc.sync.dma_start(
                        yg[(s * 3 + t) * P : (s * 3 + t + 1) * P, :], yf[:]
                    )
    nc.compile()
    return nc




def _chunks(cap):
    """384-token chunks (fewest matmuls under the 8-PSUM-bank budget); the
    runt goes last so the tail retire chain is short."""
    sizes = [384] * (cap // 384)
    rem = cap - 384 * len(sizes)
    if rem:
        sizes.append(rem)
    return sizes


def _build_fast(cap: int):
    """Fast path (b1 == 0 and b2 == 0): inputs pre-gated and pre-tiled on host.

    All matmul operands bf16 (fp32 PSUM accumulation).  f-groups (NF_G
    chunks of F) outer, token chunks inner; weights stream through SBUF
    once, per-chunk y accumulates fp32 in SBUF across groups.
      inputs : xgf [cap*1024]  gated tokens bf16, per-chunk tiled [ci, co, n]
               w1p [32, 128, 8, 128]  w1.T tiled for mm1 lhsT, bf16
               w2t [4096, 1024] bf16
      output : yg  [cap, 1024] fp32
    """
    import concourse.mybir as mybir
    from concourse import bacc
    from concourse.tile import TileContext

    f32 = mybir.dt.float32
    bf16 = mybir.dt.bfloat16
    sizes = _chunks(cap)
    offs = [sum(sizes[:i]) for i in range(len(sizes))]
    NF_G = 4
    NG = (F // P) // NF_G  # 8 groups
    nc = bacc.Bacc(None, target_bir_lowering=False)

    xgf = nc.dram_tensor("xgf", [cap * C], bf16, kind="ExternalInput")
    w1p = nc.dram_tensor("w1p", [F // P, P, 8, P], bf16, kind="ExternalInput")
    w2t = nc.dram_tensor("w2t", [F, C], bf16, kind="ExternalInput")
    # boot blob: w1[fl0] tiles ++ chunk-0 tokens, one transfer to start fast
    BOOTW = 8 * P + 8 * sizes[0]
    boot = nc.dram_tensor("boot", [P, BOOTW], bf16, kind="ExternalInput")
    yg = nc.dram_tensor("yg", [cap, C], f32, kind="ExternalOutput")

    with TileContext(nc) as tc:
        with (
            tc.tile_pool(name="ybuf", bufs=1) as ybuf,
            tc.tile_pool(name="wpool", bufs=2) as wpool,
            tc.tile_pool(name="xpool", bufs=2) as xpool,
            tc.tile_pool(name="hpool", bufs=3) as hpool,
            tc.tile_pool(name="warm", bufs=1) as warm,
            tc.tile_pool(name="psum_h", bufs=2, space="PSUM") as psum_h,
            tc.tile_pool(name="psum_y", bufs=1, space="PSUM") as psum_y,
        ):
            # HAM warmup: a burst of tiny matmuls on a zeroed scratch tile
            # keeps the PE busy while the first x/w DMAs are in flight, so
            # the clock gate lifts (1.2 -> 2.4 GHz) before real work starts
            wsb = warm.tile([P, 64], bf16, name="wsb", tag="wsb")
            nc.gpsimd.memset(wsb[:], 0)
            wps = psum_h.tile([P, 384], f32, tag="h", name="hps")
            for _ in range(76):
                nc.tensor.matmul(
                    wps[:32, :64], lhsT=wsb[:, :32], rhs=wsb[:, :64],
                    start=True, stop=True,
                )
            y_all = [
                [
                    ybuf.tile([P, C], f32, name=f"yall_{s}_{t}", tag=f"yall_{s}_{t}")
                    for t in range((sz + P - 1) // P)
                ]
                for s, sz in enumerate(sizes)
            ]

            def load_xg(s):
                sz = sizes[s]
                xg_s = xpool.tile([P, 8, sz], bf16, tag="xg", name="xg_s")
                src = xgf[offs[s] * C : (offs[s] + sz) * C]
                v = src.rearrange("(ci co n) -> ci co n", ci=P, co=8)
                nc.sync.dma_start(xg_s[:], v)
                return xg_s

            yq = 0  # rotating PSUM-y slot cursor (3 slots of 2 banks)
            for g in range(NG):
                w1g = wpool.tile([P, NF_G, 8, P], bf16, tag="w1g", name="w1g")
                w2g = wpool.tile([P, NF_G, C], bf16, tag="w2g", name="w2g")

                def load_w1(fl, g=g, w1g=w1g):
                    nc.sync.dma_start(w1g[:, fl], w1p[g * NF_G + fl])

                def load_w2(fl, g=g, w2g=w2g):
                    r0 = (g * NF_G + fl) * P
                    nc.sync.dma_start(w2g[:, fl], w2t[r0 : r0 + P, :])

                if g == 0:
                    # startup: ONE boot transfer carries w1-fl0 + all of
                    # chunk 0, so the first fl runs off a single full-rate
                    # DMA; the rest follows in pipeline-consumption order
                    boot_sb = warm.tile([P, BOOTW], bf16, name="boot_sb", tag="boot_sb")
                    nc.sync.dma_start(boot_sb[:], boot[:, :])
                    xg_next = None  # chunk 0 reads from boot_sb
                    load_w1(1)
                    load_w2(0)
                    load_w1(2)
                    load_w2(1)
                    load_w1(3)
                    load_w2(2)
                    load_w2(3)
                    load_w1(0)  # chunks 1+ still read w1g fl0 (boot covers chunk 0)
                else:
                    for fl in range(NF_G):
                        load_w1(fl)
                    for fl in range(NF_G):
                        load_w2(fl)

                for s, sz in enumerate(sizes):
                    nt = (sz + P - 1) // P
                    boot0 = g == 0 and s == 0
                    xg_s = xg_next
                    # prefetch the next chunk (wraps to s=0 of the next group)
                    if s + 1 < len(sizes):
                        xg_next = load_xg(s + 1)
                    elif g + 1 < NG:
                        xg_next = load_xg(0)

                    # rotate PSUM-y slots so consecutive chunks hit different
                    # banks (the next chunk's first mm2 must not wait on this
                    # chunk's retire reads)
                    yps = [
                        psum_y.tile(
                            [P, C], f32,
                            tag=f"y_{(yq + t) % 3}", name=f"y_{(yq + t) % 3}",
                        )
                        for t in range(nt)
                    ]
                    yq += nt

                    def mm2(fl, hT, yps=yps, nt=nt, w2g=w2g):
                        for t in range(nt):
                            for cc in range(2):
                                nc.tensor.matmul(
                                    yps[t][:, cc * 512 : (cc + 1) * 512],
                                    lhsT=hT[:, t * P : (t + 1) * P],
                                    rhs=w2g[:, fl, cc * 512 : (cc + 1) * 512],
                                    start=(fl == 0),
                                    stop=(fl == NF_G - 1),
                                )

                    last_chunk = s == len(sizes) - 1

                    def retire(s=s, g=g, yps=yps, nt=nt, last_chunk=last_chunk):
                        if g == NG - 1 and last_chunk:
                            # kernel tail: split adds per C-half and alternate
                            # DMA queues so stores pipeline with the adds
                            for t in range(nt):
                                ya = y_all[s][t]
                                w0 = offs[s] // P + t
                                for hh in range(2):
                                    sl = slice(hh * 512, (hh + 1) * 512)
                                    nc.vector.tensor_add(
                                        ya[:, sl], ya[:, sl], yps[t][:, sl]
                                    )
                                    eng = nc.scalar if (2 * t + hh) % 2 else nc.sync
                                    eng.dma_start(
                                        yg[w0 * P : (w0 + 1) * P, sl], ya[:, sl]
                                    )
                            return
                        for t in range(nt):
                            ya = y_all[s][t]
                            if g == 0:
                                nc.vector.tensor_copy(ya[:], yps[t][:])
                            else:
                                nc.vector.tensor_add(ya[:], ya[:], yps[t][:])
                            if g == NG - 1:
                                w0 = offs[s] // P + t
                                eng = nc.scalar if t % 2 else nc.sync
                                eng.dma_start(yg[w0 * P : (w0 + 1) * P, :], ya[:])

                    # software pipeline: mm2 runs one fl behind mm1 (relu
                    # latency covered by the next fl's mm1s); the last mm2 +
                    # the PSUM retires of each chunk drain after the next
                    # chunk's first mm1 block
                    hT_prev = None
                    for fl in range(NF_G):
                        hps = psum_h.tile([P, 384], f32, tag="h", name="hps")
                        for c in range(8):
                            if boot0:
                                lhsT = (
                                    boot_sb[:, c * P : (c + 1) * P]
                                    if fl == 0
                                    else w1g[:, fl, c, :]
                                )
                                rhs = boot_sb[:, 8 * P + c * sz : 8 * P + (c + 1) * sz]
                            else:
                                lhsT = w1g[:, fl, c, :]
                                rhs = xg_s[:, c, :]
                            nc.tensor.matmul(
                                hps[:, :sz],
                                lhsT=lhsT,
                                rhs=rhs,
                                start=(c == 0),
                                stop=(c == 7),
                            )
                        hT = hpool.tile([P, 384], bf16, tag="hT", name="hT")
                        if fl == NF_G - 1:
                            # last fl: per-token-tile relu so mm2(t) can
                            # start as soon as its slice is ready
                            for t in range(nt):
                                tl = slice(t * P, min((t + 1) * P, sz))
                                nc.scalar.activation(
                                    hT[:, tl],
                                    hps[:, tl],
                                    mybir.ActivationFunctionType.Relu,
                                )
                        else:
                            nc.scalar.activation(
                                hT[:, :sz],
                                hps[:, :sz],
                                mybir.ActivationFunctionType.Relu,
                            )
                        if hT_prev is not None:
                            mm2(fl - 1, hT_prev)
                        hT_prev = hT
                    mm2(NF_G - 1, hT_prev)
                    retire()
    nc.compile()
    return nc


_CACHE = {}
_TRACE = False  # test harness sets True to capture an NTFF profile
_LAST_RES = None


def _get_nc(cap, fast):
    key = (cap, fast)
    if key not in _CACHE:
        _CACHE[key] = _build_fast(cap) if fast else _build(cap)
    return _CACHE[key]


def _route(x_flat, router_w):
    """Top-2 routing, float64 for stable selection. Returns idx/weights per expert."""
    logits = x_flat.astype(np.float64) @ router_w.astype(np.float64).T
    t = np.exp(logits - logits.max(-1, keepdims=True))
    p = t / t.sum(-1, keepdims=True)
    top2 = np.argsort(-p, axis=-1)[:, :2]
    pv = np.take_along_axis(p, top2, axis=-1)
    wn = pv / (pv.sum(-1, keepdims=True) + 1e-9)
    return top2, wn


def kernel(x, router_w, w1, b1, w2, b2):
    from concourse.bass_utils import run_bass_kernel_spmd

    Bx, Nx, Cx = x.shape
    x_flat = np.ascontiguousarray(x.reshape(-1, Cx))
    T = x_flat.shape[0]

    top2, wn = _route(x_flat, router_w)
    idxs, gates = [], []
    for e in range(E):
        sel = top2 == e
        we = np.where(sel, wn, 0.0).sum(-1)
        idx = np.nonzero(sel.any(-1))[0]
        idxs.append(idx)
        gates.append(we[idx].astype(np.float32))
    cap = max(len(i) for i in idxs)
    fastcap = ((cap + P - 1) // P) * P
    cap = ((cap + SCH - 1) // SCH) * SCH

    fast = bool(np.all(b1 == 0) and np.all(b2 == 0))
    if fast:
        cap = fastcap
    nc = _get_nc(cap, fast)

    in_maps = []
    for e in range(E):
        n_e = len(idxs[e])
        xg = np.zeros((cap, Cx), np.float32)
        xg[:n_e] = x_flat[idxs[e]]
        wg = np.zeros(cap, np.float32)
        wg[:n_e] = gates[e]
        if fast:
            xg *= wg[:, None]  # pre-gate: exact since b1 == 0 and wg >= 0
            sizes = _chunks(cap)
            blocks, off = [], 0
            for sz in sizes:
                blocks.append(
                    np.ascontiguousarray(
                        xg[off : off + sz].reshape(sz, 8, P).transpose(2, 1, 0)
                    ).ravel()
                )
                off += sz
            w1p_b = np.ascontiguousarray(
                w1[e].reshape(F // P, P, 8, P).transpose(0, 3, 2, 1)
            ).astype(BF16)
            xgf_b = np.concatenate(blocks).astype(BF16)
            boot_b = np.concatenate(
                [
                    w1p_b[0].reshape(P, 8 * P),
                    xgf_b[: sizes[0] * Cx].reshape(P, 8 * sizes[0]),
                ],
                axis=1,
            )
            in_maps.append(
                {
                    "xgf": xgf_b,
                    "w1p": w1p_b,
                    "w2t": np.ascontiguousarray(w2[e].T).astype(BF16),
                    "boot": np.ascontiguousarray(boot_b),
                }
            )
        else:
            in_maps.append(
                {
                    "xgT": np.ascontiguousarray(xg.T),
                    "w1t": np.ascontiguousarray(w1[e].T),
                    "w2t": np.ascontiguousarray(w2[e].T),
                    "b1r": np.ascontiguousarray(b1[e].reshape(F // P, P).T),
                    "b2r": np.ascontiguousarray(np.broadcast_to(b2[e], (P, Cx))),
                    "wg": np.ascontiguousarray(wg.reshape(cap // P, P).T),
                }
            )

    global _LAST_RES
    res = run_bass_kernel_spmd(nc, in_maps, core_ids=list(range(E)), trace=_TRACE)
    _LAST_RES = res

    out = np.zeros((T, Cx), np.float32)
    for e in range(E):
        n_e = len(idxs[e])
        out[idxs[e]] += res.results[e]["yg"][:n_e]
    return out.reshape(Bx, Nx, Cx)
